# revision 51
# baseline (speedup 1.0000x reference)
"""Trainium2 Bass kernel for nn_DecoderRNN (LSTM decoder + big vocab projection).

Strategy (8 NeuronCores, SPMD):
  - LSTM recurrence (B=32, T=64, H=512) replicated on every core (its cost is
    batch-independent); output projection fc tensor-parallel over vocab:
    core c writes logits[:, :, 1250c:1250(c+1)], host concatenates.
  - Step t's gate pre-activations accumulate in a [32, 512] PSUM bank per
    gate chunk from ONE fp8 DoubleRow group with K=1024:
        xs(t) @ W_ih.T (2 DR matmuls)  +  h(t-1) @ W_hh.T (2 DR matmuls).
    The xs matmuls don't depend on h(t-1): they are emitted at the end of
    step t-1 and stream through the PE while the nonlinear tail runs.  This
    dense real work keeps the PE HAM clock gate at 2.4 GHz (the PE throttles
    to 1.2 GHz whenever it idles through a ~3.4us activity window, and sparse
    schedules here measured 2x slower per matmul).
  - Bias trick: xs columns 510/511 are overwritten with the constant 1.0
    (x16 in fp8) on device, and W_ih.T rows 510/511 carry 32*(b_ih + b_hh),
    so the bias rides the xs matmuls for free.  The lost true xs[510:512]
    contribution is negligible for embedding rows; for the t=0 features row
    it is restored exactly by a rank-2 correction matmul in the prologue.
  - Scaling: xsT8/hsT8 = 16x values, W weights = 64x -> gate PSUM = 1024x
    true; activations descale by 1/1024.
  - Gate chunk order in SBUF columns is [f | i | g | o].  Scalar act queue
    (f0, i, g, f1, o0, o1): f0 heads the f*c chain, i/g feed the ig product,
    f1's consumer (f*c half 1) runs on the otherwise-idle gpsimd.
  - Cell state c is bf16 (2x-rate DVE ops; error is negligible vs the fp8
    recurrence quantization).  fc drains emit bf16, the host upcasts.

PSUM budget (8 banks): 5 gate banks (4 live + staging for next step's xs
groups) + 2 fc banks + 1 transpose bank.

kernel(**inputs) takes FULL unsharded inputs, returns FULL [32, 64, 10000] f32.
"""

import sys

sys.path.insert(0, "/opt/trn_rl_repo")

import numpy as np

N_CORES = 8
B, T = 32, 64
E, H, V = 512, 512, 10000
G4 = 4 * H            # 2048
TB = T * B            # 2048
VSL = V // N_CORES    # 1250 vocab rows per core
VPAD = 1280           # padded so fc N-chunks are 512/512/256 (all >=256)

_PROGRAM = None


def _build_program():
    import concourse.bass as bass
    import concourse.tile as tile
    from concourse import bacc, mybir
    from concourse.masks import make_identity
    from contextlib import ExitStack

    f32 = mybir.dt.float32
    bf16 = mybir.dt.bfloat16
    f8e4 = mybir.dt.float8e4
    i32 = mybir.dt.int32
    AF = mybir.ActivationFunctionType
    DR = mybir.MatmulPerfMode.DoubleRow

    nc = bacc.Bacc(
        "TRN2",
        target_bir_lowering=False,
        debug=False,
        num_devices=N_CORES,
    )

    features = nc.dram_tensor("features", [B, E], bf16, kind="ExternalInput").ap()
    idx = nc.dram_tensor("idx", [TB], i32, kind="ExternalInput").ap()
    embed = nc.dram_tensor("embed", [V, E], f32, kind="ExternalInput").ap()
    # weights arrive host-pre-transposed into the SBUF pair layout so the
    # prologue DMAs are fully contiguous per partition
    wih8 = nc.dram_tensor("wih8", [128, 2, 2, G4], f8e4, kind="ExternalInput").ap()
    whh8 = nc.dram_tensor("whh8", [128, 2, 2, G4], f8e4, kind="ExternalInput").ap()
    ftail = nc.dram_tensor("ftail", [2, B], bf16, kind="ExternalInput").ap()
    wtail = nc.dram_tensor("wtail", [2, G4], bf16, kind="ExternalInput").ap()
    fcwT = nc.dram_tensor("fcwT", [128, 4, VPAD], bf16, kind="ExternalInput").ap()
    fcb = nc.dram_tensor("fcb", [VPAD], bf16, kind="ExternalInput").ap()
    onesv = nc.dram_tensor("onesv", [128], bf16, kind="ExternalInput").ap()
    out = nc.dram_tensor("out", [B, T, VSL], bf16, kind="ExternalOutput").ap()
    # Output viewed as [t, b, v]: a 128-row t-major tb tile = 4 t planes.
    out_r = out.rearrange("b t v -> t b v")

    with tile.TileContext(nc) as tc, ExitStack() as ctx:
        # ---------------- persistent state ----------------
        state = ctx.enter_context(tc.tile_pool(name="state", bufs=1))
        # h.T history: block t holds h(t).T (written at the end of step t).
        # Layout [p, k, 32*t + b] = h(t)[b, 128*k + p]
        hsT = state.tile([128, 4, 32 * T], bf16, tag="hsT")
        # fp8 16x copies for the DoubleRow matmuls: pair layout
        # [p, P, i, .] = contraction row 128*(2P+i)+p.
        hsT8 = state.tile([128, 2, 2, 32 * T], f8e4, tag="hsT8")
        xsT8 = state.tile([128, 2, 2, TB], f8e4, tag="xsT8")
        whh8_sb = state.tile([128, 2, 2, G4], f8e4, tag="whh8")
        wih8_sb = state.tile([128, 2, 2, G4], f8e4, tag="wih8")
        ftail_sb = state.tile([2, B], bf16, tag="ftail")
        wtail_sb = state.tile([2, G4], bf16, tag="wtail")
        fcwT_sb = state.tile([128, 4, VPAD], bf16, tag="fcwT")
        fcb_sb = state.tile([1, VPAD], bf16, tag="fcb")
        fcb128 = state.tile([128, VPAD], f32, tag="fcb128")
        c_sb = state.tile([B, H], bf16, tag="c")
        ident_b = state.tile([128, 128], bf16, tag="ident_b")
        ones = state.tile([1, 128], bf16, tag="ones")
        idx_sb = state.tile([128, 16], i32, tag="idx")

        make_identity(nc, ident_b[:])
        nc.vector.memset(c_sb[:], 0.0)

        # ---------------- PSUM pools ----------------
        g_psum = ctx.enter_context(tc.tile_pool(name="g_ps", bufs=5, space="PSUM"))
        fc_psum = ctx.enter_context(tc.tile_pool(name="fc_ps", bufs=2, space="PSUM"))
        h_psum = ctx.enter_context(tc.tile_pool(name="h_ps", bufs=1, space="PSUM"))

        gate_tiles = {}

        def emit_xs(t):
            """Open step t's four chunk groups with xs(t) @ W_ih.T (+ bias via
            the constant columns).  Independent of h(t-1) -> streams during
            step t-1's tail, keeping the PE hot."""
            for cch in range(4):
                sl = slice(512 * cch, 512 * (cch + 1))
                gt = g_psum.tile([B, 512], f32, tag="g")
                gate_tiles[(t, cch)] = gt
                for P in (0, 1):
                    nc.tensor.matmul(
                        gt[:],
                        lhsT=xsT8[:, P, :, 32 * t : 32 * (t + 1)],
                        rhs=wih8_sb[:, P, :, sl],
                        start=(P == 0),
                        stop=False,
                        perf_mode=DR,
                        skip_group_check=True,
                    )
                if t == 0:
                    # restore the features[:, 510:512] contribution displaced
                    # by the constant bias columns (exact rank-2 correction)
                    nc.tensor.matmul(
                        gt[:], lhsT=ftail_sb[:, :], rhs=wtail_sb[:, sl],
                        start=False, stop=True, skip_group_check=True,
                    )

        # xs tiles outlive the prologue (in-loop transposes read them); one
        # tile per 128-row block so a transpose only waits on its own gather.
        xs_pool = ctx.enter_context(tc.tile_pool(name="xs", bufs=1))
        xs_tiles = [
            xs_pool.tile([128, E], bf16, tag=f"xs_b{mm}", name=f"xs_b{mm}")
            for mm in range(16)
        ]

        def transpose_m(m):
            """xs tile m -> xsT8 (fp8, 16x) via PE transpose + scaled copy."""
            for e in range(4):
                pt = h_psum.tile([128, 128], bf16, tag="hp")
                nc.tensor.transpose(
                    pt[:], xs_tiles[m][:, 128 * e : 128 * (e + 1)], ident_b[:]
                )
                nc.vector.tensor_scalar_mul(
                    xsT8[:, e // 2, e % 2, 128 * m : 128 * (m + 1)], pt[:], 16.0
                )

        # ---------------- prologue: gather + weight loads ----------------
        with ExitStack() as pro:
            nc.sync.dma_start(idx_sb[:], idx.rearrange("(m p) -> p m", p=128))
            nc.sync.dma_start(wih8_sb[:], wih8[:, :, :, :])
            nc.sync.dma_start(whh8_sb[:], whh8[:, :, :, :])
            nc.sync.dma_start(fcwT_sb[:], fcwT[:, :, :])
            nc.sync.dma_start(fcb_sb[:], fcb[None, :])
            nc.sync.dma_start(ones[:], onesv[None, :])
            nc.sync.dma_start(ftail_sb[:], ftail[:, :])
            nc.sync.dma_start(wtail_sb[:], wtail[:, :])
            # fcb128 = broadcast(fc_b) via rank-1 matmuls into the fc bank
            for c0, csz in ((0, 512), (512, 512), (1024, 256)):
                bp = fc_psum.tile([128, 512], f32, tag="fc")
                nc.tensor.matmul(bp[:, 0:csz], lhsT=ones[0:1, :],
                                 rhs=fcb_sb[0:1, c0 : c0 + csz], start=True, stop=True)
                nc.vector.tensor_copy(fcb128[:, c0 : c0 + csz], bp[:, 0:csz])

            def gather(m):
                nc.gpsimd.indirect_dma_start(
                    out=xs_tiles[m][:, :],
                    out_offset=None,
                    in_=embed[:, :],
                    in_offset=bass.IndirectOffsetOnAxis(
                        ap=idx_sb[:, m : m + 1], axis=0
                    ),
                )
                if m == 0:
                    nc.sync.dma_start(xs_tiles[0][0:32, :], features[:, :])
                # constant bias-carrier columns (x16 by the transpose copy)
                nc.vector.memset(xs_tiles[m][:, 510:512], 1.0)

            gather(0)
            gather(1)
            # Warm-up burst: the PE would otherwise idle through the serial
            # embedding gathers, so step 0 would start with the HAM clock
            # gate throttled to 1.2 GHz.  ~5us of dummy matmuls here (fully
            # overlapped with the gathers) un-throttle it for free.
            for wmm in range(48):
                wp = fc_psum.tile([128, 512], f32, tag="fc", name="warm")
                nc.tensor.matmul(
                    wp[:, 0:128], lhsT=ident_b[:], rhs=ident_b[:, 0:128],
                    start=True, stop=True, skip_group_check=True,
                )
            transpose_m(0)
            transpose_m(1)
            emit_xs(0)
            for m in range(2, 16):
                gather(m)

        # ---------------- main recurrence + interleaved fc ----------------
        work = ctx.enter_context(tc.tile_pool(name="work", bufs=3))
        lg_pool = ctx.enter_context(tc.tile_pool(name="lg", bufs=2))

        FC_CHUNKS = ((0, 512), (512, 512), (1024, 256))
        lg_tiles = {}

        def fc_chunk_mms(m, j):
            """PE part of fc chunk j for tb tile m (fills PE bubbles)."""
            if j == 0:
                lg_new = lg_pool.tile([128, VPAD], bf16, tag="lg")
                lg_tiles[m] = lg_new
            c0, csz = FC_CHUNKS[j]
            fps = fc_psum.tile([128, 512], f32, tag="fc")
            for k in range(4):
                nc.tensor.matmul(
                    fps[:, 0:csz],
                    lhsT=hsT[:, k, 128 * m : 128 * (m + 1)],
                    rhs=fcwT_sb[:, k, c0 : c0 + csz],
                    start=(k == 0),
                    stop=(k == 3),
                )
            return fps

        def fc_chunk_finish(m, j, fps):
            c0, csz = FC_CHUNKS[j]
            nc.vector.tensor_add(
                lg_tiles[m][:, c0 : c0 + csz], fps[:, 0:csz], fcb128[:, c0 : c0 + csz]
            )
            if j == 2:
                # DRAM side is [4 t, 32 b, 1250 v]; SBUF side [128, 1250]
                # pairs element-stream-wise (partition p = 32*t_local + b).
                nc.sync.dma_start(
                    out_r[4 * m : 4 * (m + 1), :, :], lg_tiles[m][:, 0:VSL]
                )

        # gate chunk order in SBUF columns (host permutes): 0=f 1=i 2=g 3=o
        for t in range(T):
            q = t % 4
            m = t // 4
            nl = work.tile([B, G4], bf16, tag="nl")

            # ---- close the chunk groups with h(t-1) @ W_hh.T ----
            # Wave order: ALL P0 matmuls (which need only hsT8 half 0 of the
            # previous step) first, then the P1 wave.  Half 1 of the previous
            # tail lags half 0 by >1us; the P0 wave streams during that lag.
            if t > 0:
                for P in (0, 1):
                    for cch in range(4):
                        gt = gate_tiles[(t, cch)]
                        nc.tensor.matmul(
                            gt[:],
                            lhsT=hsT8[:, P, :, 32 * (t - 1) : 32 * t],
                            rhs=whh8_sb[:, P, :, 512 * cch : 512 * (cch + 1)],
                            start=False,
                            stop=(P == 1),
                            perf_mode=DR,
                            skip_group_check=True,
                        )

            # Scalar act queue order (f0, i, g, f1, o0, o1): f0 heads the
            # f*c chain; i and g feed sigmoid(i)*tanh(g) next; f1 (whose
            # consumer runs on gpsimd) is deferred behind g.
            def act(cch, ah=None):
                g_tile = gate_tiles[(t, cch)]
                fn = AF.Tanh if cch == 2 else AF.Sigmoid
                if ah is None:
                    nc.scalar.activation(
                        nl[:, 512 * cch : 512 * (cch + 1)], g_tile[:],
                        fn, scale=1.0 / 1024.0,
                    )
                else:
                    nc.scalar.activation(
                        nl[:, 512 * cch + 256 * ah : 512 * cch + 256 * (ah + 1)],
                        g_tile[:, 256 * ah : 256 * (ah + 1)],
                        fn, scale=1.0 / 1024.0,
                    )

            act(0, 0)
            act(1)
            act(2)
            act(0, 1)
            act(3)

            # ---- PE fillers ----
            fc_pending = None
            if q < 3 and m >= 1:
                fc_pending = fc_chunk_mms(m - 1, q)

            # ---- c/h update, halves pipelined ----
            # c = sigmoid(f)*c + sigmoid(i)*tanh(g);  h = sigmoid(o)*tanh(c)
            fmul = work.tile([B, H], bf16, tag="fmul")
            ig = work.tile([B, H], bf16, tag="ig")
            tanhc = work.tile([B, H], bf16, tag="tanhc")
            h_t = work.tile([B, H], bf16, tag="h")
            nc.vector.tensor_mul(fmul[:, 0:256], nl[:, 0:256], c_sb[:, 0:256])
            nc.vector.tensor_mul(fmul[:, 256:512], nl[:, 256:512], c_sb[:, 256:512])
            nc.vector.tensor_mul(ig[:], nl[:, 512:1024], nl[:, 1024:1536])
            hp = h_psum.tile([128, 128], bf16, tag="hp")
            for half in (0, 1):
                hs = slice(256 * half, 256 * (half + 1))
                nc.vector.tensor_add(c_sb[:, hs], fmul[:, hs], ig[:, hs])
                nc.scalar.activation(tanhc[:, hs], c_sb[:, hs], AF.Tanh)
                nc.vector.tensor_mul(
                    h_t[:, hs], nl[:, 1536 + 256 * half : 1536 + 256 * (half + 1)],
                    tanhc[:, hs],
                )
                for k in (2 * half, 2 * half + 1):
                    nc.tensor.transpose(
                        hp[:, 32 * k : 32 * (k + 1)],
                        h_t[0:32, 128 * k : 128 * (k + 1)],
                        ident_b[0:32, 0:32],
                    )
                # fp8 copy (16*h.T) first: it gates the next step's DoubleRow
                # matmuls; the bf16 hsT copy (fc input) can lag.  half
                # doubles as the pair index P (k = 2*P + i).
                nc.vector.tensor_scalar_mul(
                    hsT8[:, half, :, 32 * t : 32 * (t + 1)],
                    hp[:, 64 * half : 64 * (half + 1)].rearrange(
                        "p (k b) -> p k b", k=2
                    ),
                    16.0,
                )
                nc.vector.tensor_copy(
                    hsT[:, 2 * half : 2 * half + 2, 32 * t : 32 * (t + 1)],
                    hp[:, 64 * half : 64 * (half + 1)].rearrange(
                        "p (k b) -> p k b", k=2
                    ),
                )

            # ---- non-critical work after the tail ----
            if fc_pending is not None:
                fc_chunk_finish(m - 1, q, fc_pending)
            if 2 + t <= 15:
                transpose_m(2 + t)
            # open next step's chunk groups: the xs matmuls stream while the
            # hsT8 copies land.
            if t + 1 < T:
                emit_xs(t + 1)

        for j in range(3):
            fps = fc_chunk_mms(15, j)
            fc_chunk_finish(15, j, fps)

    nc.compile()
    return nc


def _get_program():
    global _PROGRAM
    if _PROGRAM is None:
        _PROGRAM = _build_program()
    return _PROGRAM


# PyTorch LSTM gate order is [i, f, g, o]; we reorder rows to [f, i, g, o] so
# the f-sigmoid (head of the c-chain) is the first chunk to complete.
def _gate_perm():
    return np.concatenate(
        [np.arange(H, 2 * H), np.arange(0, H), np.arange(2 * H, 3 * H), np.arange(3 * H, 4 * H)]
    )


def _make_in_maps(features, captions, embed_table, W_ih, W_hh, b_ih, b_hh, fc_W, fc_b):
    import ml_dtypes

    bf16 = ml_dtypes.bfloat16
    f8e4 = ml_dtypes.float8_e4m3
    perm = _gate_perm()
    features = np.asarray(features, dtype=np.float32)
    features_b = np.ascontiguousarray(features.astype(bf16))
    cap = np.asarray(captions).astype(np.int32)                      # [B, T]
    embed = np.ascontiguousarray(np.asarray(embed_table, dtype=np.float32))
    wihT_p = np.asarray(W_ih, dtype=np.float32)[perm].T              # [E, 4H]
    bsum = (np.asarray(b_ih, dtype=np.float32) + np.asarray(b_hh, dtype=np.float32))[perm]
    # input weights fp8 x64; rows 510/511 become the bias carriers (the xs
    # side holds constant 16.0 there): 2 * 16 * 32b = 1024b
    wih8_f = wihT_p * 64.0
    wih8_f[510:512, :] = bsum * 32.0
    # pre-transpose to the SBUF pair layout [p, P, i, g]
    wih8 = np.ascontiguousarray(
        np.clip(wih8_f, -240, 240).astype(f8e4)
        .reshape(2, 2, 128, G4).transpose(2, 0, 1, 3)
    )
    # exact rank-2 correction for the displaced features[:, 510:512] @ rows
    # (t=0 only): (16 f) @ (64 w) = 1024 * (f @ w)
    ftail = np.ascontiguousarray((features[:, 510:512].T * 16.0).astype(bf16))
    wtail = np.ascontiguousarray((wihT_p[510:512, :] * 64.0).astype(bf16))
    whh8 = np.ascontiguousarray(
        np.clip(np.asarray(W_hh, dtype=np.float32)[perm].T * 64.0, -240, 240)
        .astype(f8e4).reshape(2, 2, 128, G4).transpose(2, 0, 1, 3)
    )
    fc_W = np.asarray(fc_W, dtype=np.float32)
    fc_b = np.asarray(fc_b, dtype=np.float32)

    # gather indices, t-major: xs row t*32+b = embed[captions[b, t-1]] for t>=1
    idx = np.zeros(TB, dtype=np.int32)
    idx[B:] = cap[:, : T - 1].T.reshape(-1)

    in_maps = []
    for c in range(N_CORES):
        sl = slice(VSL * c, VSL * (c + 1))
        fcwT = np.zeros((H, VPAD), dtype=bf16)
        fcwT[:, :VSL] = fc_W[sl].T.astype(bf16)
        fcwT = fcwT.reshape(4, 128, VPAD).transpose(1, 0, 2)
        fcbp = np.zeros(VPAD, dtype=bf16)
        fcbp[:VSL] = fc_b[sl].astype(bf16)
        in_maps.append(
            dict(
                features=features_b,
                idx=idx,
                embed=embed,
                wih8=wih8,
                whh8=whh8,
                ftail=ftail,
                wtail=wtail,
                fcwT=np.ascontiguousarray(fcwT),
                fcb=fcbp,
                onesv=np.ones(128, dtype=bf16),
            )
        )
    return in_maps


def _install_ntff_hook():
    """Wire up NTFF profiling: bass_utils wants antenv.axon_hooks, which this
    container lacks; build it from trn_agent_boot's ctypes hook."""
    import sys as _sys
    import types

    if "antenv.axon_hooks" in _sys.modules:
        return
    if "/root/.axon_site" not in _sys.path:
        _sys.path.insert(0, "/root/.axon_site")
    from trn_agent_boot.trn_boot import _ntff_profile_via_ctypes

    hook = _ntff_profile_via_ctypes("/opt/axon/libaxon_pjrt.so")
    mod = types.ModuleType("antenv.axon_hooks")
    mod._hook = hook
    mod.set_axon_ntff_profile_hook = lambda h: setattr(mod, "_hook", h)
    mod.get_axon_ntff_profile_hook = lambda: mod._hook
    _sys.modules["antenv.axon_hooks"] = mod

    # avoid S3 uploads from the trace path in this zero-egress container
    import concourse.bass_utils as bu

    bu.upload_artifacts = lambda tmpdir: f"local:{tmpdir}"


def run(inputs, trace=False, trace_cores=None):
    """Run on hardware; returns (full_output [B,T,V] f32, BassKernelResults)."""
    from concourse.bass_utils import run_bass_kernel_spmd

    if trace:
        _install_ntff_hook()

    nc = _get_program()
    in_maps = _make_in_maps(
        inputs["features"],
        inputs["captions"],
        inputs["embed_table"],
        inputs["W_ih"],
        inputs["W_hh"],
        inputs["b_ih"],
        inputs["b_hh"],
        inputs["fc_W"],
        inputs["fc_b"],
    )
    kwargs = {}
    if trace:
        import os
        import shutil

        shutil.rmtree("/tmp/bass_trace", ignore_errors=True)
        os.makedirs("/tmp/bass_trace", exist_ok=True)
        kwargs.update(trace=True, trace_cores=trace_cores or [0], tmpdir="/tmp/bass_trace")
    res = run_bass_kernel_spmd(nc, in_maps, core_ids=list(range(N_CORES)), **kwargs)
    full = np.concatenate(
        [np.asarray(r["out"]).astype(np.float32) for r in res.results], axis=2
    )
    return full, res


def kernel(**inputs) -> np.ndarray:
    out, _ = run(inputs, trace=False)
    return out
